# revision 8
# baseline (speedup 1.0000x reference)
"""AttnBlock (GroupNorm -> QKV 1x1 -> HWxHW attention -> proj -> residual)
for Trainium2, data-parallel over batch across 8 NeuronCores.

Full inputs in, full outputs out. Internally:
  - batch B=32 is split 4 images per core
  - per image (C=512, N=H*W=1024):
      h = groupnorm(x)                                  [C, N] fp32r
      q = wq@h + bq    (CN layout, fp32r)               [C, N]
      k = wk@h + bk    (CN layout, fp32r)               [C, N]
      sT = k^T q       (scores transposed [nk, nq], fp32r: exact logits)
      -- everything downstream of the logits runs fp8(e4m3) DoubleRow
         matmuls at 2 fp8 weights/PE-cell (contract 256/matmul):
      h8  = fp8(8*h)        (from x via scalar activation)
      vT8 = fp8(16*v)       (NC layout; v = wv@h via fp8 DR matmul)
      eT8 = fp8(exp(alpha*sT - 4ln2))  ( = exp-scores / 16 )
      Z16 = ones^T eT8      (col sums = Z/16, PE-broadcast, DR matmul)
      oT8 = fp8((vT8^T eT8) * (1/Z16))  ( = 16*attn_out, DR matmul )
      y = x + wp8@oT8 / 4096 + bp'      (DR matmul; bp' = wp@bv + bp)
  fp8 scales keep all tensors well under the TRN e4m3 +-240 limit.
  Logit path stays fp32r so softmax weights are exact; fp8 error sites
  (h8, wv8, vT8, eT8, oT8, wp8) each contribute ~0.5% to the global
  max-rel metric, ~1.2% total (tolerance 2e-2).
"""

import numpy as np

P = 128
C = 512
KC = C // P          # 4 chunks of channels
N = 1024             # H*W
NB = N // P          # 8 blocks of pixels
FD = 512             # matmul moving free dim (one PSUM bank of fp32)
NH = N // FD         # 2 halves of the query axis
IMGS = 4             # images per core
NCORES = 8
G8 = 8               # groups per 128-channel tile (group size 16)
GS = 16
ALPHA = float(C) ** -0.5
EPS = 1e-6
EXP_SHIFT = 4.0 * float(np.log(2.0))   # eT8 = exp(alpha*s)/16
S_H = 8.0            # h8 = 8*h
S_W = 256.0          # wv8 = 256*wv, wp8 = 256*wp

_CACHE = {}


def build_nc():
    """Build the single-core Bass/Tile program (SPMD across 8 cores)."""
    import concourse.bacc as bacc
    import concourse.mybir as mybir
    import concourse.tile as tile
    from concourse.bass import ts

    f32 = mybir.dt.float32
    f32r = mybir.dt.float32r
    f8 = mybir.dt.float8e4
    AF = mybir.ActivationFunctionType
    OP = mybir.AluOpType
    DR = mybir.MatmulPerfMode.DoubleRow

    nc = bacc.Bacc("TRN2", target_bir_lowering=False, debug=False)

    x_t = nc.dram_tensor("x", [IMGS, C, N], f32, kind="ExternalInput")
    wqT_t = nc.dram_tensor("wqT", [C, C], f32r, kind="ExternalInput")
    wkT_t = nc.dram_tensor("wkT", [C, C], f32r, kind="ExternalInput")
    wv8T_t = nc.dram_tensor("wv8T", [C, C], f8, kind="ExternalInput")
    wp8T_t = nc.dram_tensor("wp8T", [C, C], f8, kind="ExternalInput")
    aux_t = nc.dram_tensor("aux", [P, 32], f32, kind="ExternalInput")
    gt8_t = nc.dram_tensor("gt8", [P, P], f32, kind="ExternalInput")
    one8_t = nc.dram_tensor("ones8", [P, 2 * P], f8, kind="ExternalInput")
    out_t = nc.dram_tensor("out", [IMGS, C, N], f32, kind="ExternalOutput")

    x_ap = x_t.ap()
    out_ap = out_t.ap()

    with tile.TileContext(nc) as tc:
        with (
            tc.tile_pool(name="const", bufs=1) as cst,
            tc.tile_pool(name="xp", bufs=3) as xp,
            tc.tile_pool(name="hp", bufs=2) as hp,
            tc.tile_pool(name="hp8", bufs=2) as hp8,
            tc.tile_pool(name="qp", bufs=1) as qp,
            tc.tile_pool(name="kp", bufs=1) as kp,
            tc.tile_pool(name="vp", bufs=1) as vp,
            tc.tile_pool(name="ep", bufs=1) as epool,
            tc.tile_pool(name="opool", bufs=1) as opool,
            tc.tile_pool(name="rp", bufs=2) as rp,
            tc.tile_pool(name="tp", bufs=4) as tp,
            tc.tile_pool(name="gn", bufs=2) as gnp,
            tc.tile_pool(name="psmm", bufs=5, space="PSUM") as psmm,
            tc.tile_pool(name="psz", bufs=1, space="PSUM") as psz,
            tc.tile_pool(name="psgn", bufs=1, space="PSUM") as psgn,
        ):
            # ---- per-image x loads: split per channel-chunk so GroupNorm
            # stats can start as soon as the first chunk lands. x(0) is
            # emitted before everything else; consts/weights go on the
            # GpSimd (SWDGE) queue so the two DMA streams overlap.
            x_tiles = {}

            def load_x(img, split=False):
                t = xp.tile([P, KC, N], f32, tag="x")
                src = x_ap[img].rearrange("(kc p) n -> p kc n", p=P)
                if split:
                    # first image: fan out over 4 queues, half-N pieces, so
                    # GroupNorm stats start as early as possible
                    engs = (nc.sync, nc.gpsimd, nc.scalar)
                    i = 0
                    for kc in range(KC):
                        for hf in range(2):
                            engs[i % 3].dma_start(
                                out=t[:, kc, ts(hf, 512)],
                                in_=src[:, kc, ts(hf, 512)],
                            )
                            i += 1
                else:
                    for kc in range(KC):
                        eng = nc.sync if kc % 2 == 0 else nc.gpsimd
                        eng.dma_start(out=t[:, kc], in_=src[:, kc])
                x_tiles[img] = t

            # ---- constants (sync queue; aux first, it is tiny and gates
            # the PE warm-up + GroupNorm chain) ----
            def cdma(shape, dt_, src_ap, tag=None, eng=None):
                t = cst.tile(shape, dt_, tag=tag)
                (eng or nc.gpsimd).dma_start(out=t, in_=src_ap)
                return t

            aux_sb = cdma([P, 32], f32, aux_t.ap(), tag="aux", eng=nc.sync)
            load_x(0, split=True)
            gt8_sb = cdma([P, P], f32, gt8_t.ap(), tag="gt8")
            w_sb = {}
            for name, t in (("q", wqT_t), ("k", wkT_t)):
                w_sb[name] = cdma(
                    [P, KC, C], f32r,
                    t.ap().rearrange("(kc p) o -> p kc o", p=P), tag=f"w{name}",
                )
            for name, t in (("v", wv8T_t), ("p", wp8T_t)):
                w_sb[name] = cdma(
                    [P, KC, C], f8,
                    t.ap().rearrange("(kc p) o -> p kc o", p=P), tag=f"w{name}",
                )
            one8_sb = cdma([P, 2, P], f8, one8_t.ap().rearrange("p (two q) -> p two q", two=2), tag="one8")
            bq_sb = aux_sb[:, 0:4]
            bk_sb = aux_sb[:, 4:8]
            bp_sb = aux_sb[:, 8:12]
            gnw_sb = aux_sb[:, 12:16]
            gnb_sb = aux_sb[:, 16:20]
            gsel_sb = aux_sb[:, 20:28]
            eps_sb = aux_sb[:, 28:29]
            expb_sb = aux_sb[:, 29:30]
            zero_sb = aux_sb[:, 30:31]

            # ---- PE warm-up: tiny matmuls keep the HAM activity monitor
            # busy through the GroupNorm latency chain so the first real
            # matmuls run at full clock (otherwise ~3.4us at half rate).
            wps = psmm.tile([G8, G8], f32, tag="warm", bufs=1)
            for _ in range(160):
                nc.tensor.matmul(wps, gsel_sb, gsel_sb, start=True, stop=True)

            h_tiles = {}
            h8_tiles = {}
            ab8_tiles = {}

            def gn(img):
                x_sb = x_tiles[img]
                # ================= GroupNorm =================
                stats = gnp.tile([P, KC, 2, 6], f32, tag="stats")
                for kc in range(KC):
                    for hf in range(2):
                        nc.vector.bn_stats(
                            out=stats[:, kc, hf], in_=x_sb[:, kc, ts(hf, 512)]
                        )
                # mv3: per-channel [mean, var, mean^2]
                mv3 = gnp.tile([P, KC, 3], f32, tag="mv3")
                for kc in range(KC):
                    nc.vector.bn_aggr(out=mv3[:, kc, 0:2], in_=stats[:, kc])
                nc.vector.tensor_mul(mv3[:, :, 2], mv3[:, :, 0], mv3[:, :, 0])
                nc.vector.tensor_add(mv3[:, :, 1], mv3[:, :, 1], mv3[:, :, 2])
                # group sums over 16-channel groups (via PE)
                gsum = psgn.tile([G8, KC, 2], f32, tag="gn")
                nc.tensor.matmul(gsum, gsel_sb, mv3[:, :, 0:2], start=True, stop=True)
                # bcin = [mean_g, std_g]
                bcin = gnp.tile([G8, KC, 2], f32, tag="bcin")
                nc.vector.tensor_scalar_mul(bcin[:, :, 0], gsum[:, :, 0], 1.0 / GS)
                tvar = gnp.tile([G8, KC], f32, tag="tvar")
                nc.vector.tensor_scalar_mul(tvar, gsum[:, :, 1], 1.0 / GS)
                m2g = gnp.tile([G8, KC], f32, tag="m2g")
                nc.vector.tensor_mul(m2g, bcin[:, :, 0], bcin[:, :, 0])
                nc.vector.tensor_sub(tvar, tvar, m2g)
                nc.scalar.activation(bcin[:, :, 1], tvar, AF.Sqrt, bias=eps_sb[:G8])
                # broadcast group stats back to channels (via PE)
                bcast = psgn.tile([P, KC, 2], f32, tag="gn")
                nc.tensor.matmul(bcast, gt8_sb[:G8], bcin, start=True, stop=True)
                # h = x * A + B; A = gn_w/std, B = gn_b - mean*A
                rstd = gnp.tile([P, KC], f32, tag="rstd")
                nc.vector.reciprocal(rstd, bcast[:, :, 1])
                a_sb = gnp.tile([P, KC], f32, tag="A")
                nc.vector.tensor_mul(a_sb, rstd, gnw_sb)
                b_sb = gnp.tile([P, KC], f32, tag="B")
                nc.vector.tensor_mul(b_sb, bcast[:, :, 0], a_sb)
                nc.vector.tensor_sub(b_sb, gnb_sb, b_sb)
                h_sb = hp.tile([P, KC, N], f32r, tag="h")
                for kc in range(KC):
                    nc.vector.tensor_scalar(
                        out=h_sb[:, kc],
                        in0=x_sb[:, kc],
                        scalar1=a_sb[:, kc : kc + 1],
                        scalar2=b_sb[:, kc : kc + 1],
                        op0=OP.mult,
                        op1=OP.add,
                    )
                h_tiles[img] = h_sb
                # scale/bias for the deferred h8 = fp8(8*h) production
                # (emitted later so the 4 big scalar ops never sit in the
                # scalar FIFO ahead of the attention exps)
                a8_sb = gnp.tile([P, KC], f32, tag="A8")
                nc.vector.tensor_scalar_mul(a8_sb, a_sb, S_H)
                b8_sb = gnp.tile([P, KC], f32, tag="B8")
                nc.vector.tensor_scalar_mul(b8_sb, b_sb, S_H)
                ab8_tiles[img] = (a8_sb, b8_sb)

            def emit_h8(img):
                # h8 = fp8(8*h) straight from x (scalar engine)
                x_sb = x_tiles[img]
                a8_sb, b8_sb = ab8_tiles.pop(img)
                h8_sb = hp8.tile([P, KC, N], f8, tag="h8")
                for kc in range(KC):
                    nc.scalar.activation(
                        h8_sb[:, kc],
                        x_sb[:, kc],
                        AF.Identity,
                        scale=a8_sb[:, kc : kc + 1],
                        bias=b8_sb[:, kc : kc + 1],
                    )
                h8_tiles[img] = h8_sb

            gn(0)
            # Deferred final-projection groups: each is emitted interleaved
            # into the NEXT matmul phase (scores of the other query half, or
            # qk of the next image) so the PE has independent work while the
            # oT8 chunks are still being produced by the vector engine.
            pending = []

            def drain(n):
                for _ in range(min(n, len(pending))):
                    pending.pop(0)()

            for img in range(IMGS):
                x_sb = x_tiles[img]
                h_sb = h_tiles.pop(img)

                # ================= q, k projections (CN layout, fp32r) ======
                q_sb = qp.tile([P, KC, N], f32r, tag="q")
                k_sb = kp.tile([P, KC, N], f32r, tag="k")
                for dst, w, bias, use_act in (
                    (q_sb, w_sb["q"], bq_sb, True),
                    (k_sb, w_sb["k"], bk_sb, False),
                ):
                    for mo in range(KC):
                        for nh in range(NH):
                            drain(1)
                            ps = psmm.tile([P, FD], f32, tag="mm")
                            for kc in range(KC):
                                nc.tensor.matmul(
                                    ps,
                                    w[:, kc, ts(mo, P)],
                                    h_sb[:, kc, ts(nh, FD)],
                                    start=(kc == 0),
                                    stop=(kc == KC - 1),
                                )
                            if use_act:
                                nc.scalar.activation(
                                    dst[:, mo, ts(nh, FD)],
                                    ps,
                                    AF.Identity,
                                    bias=bias[:, mo : mo + 1],
                                )
                            else:
                                nc.vector.tensor_scalar_add(
                                    dst[:, mo, ts(nh, FD)],
                                    ps,
                                    bias[:, mo : mo + 1],
                                )

                if img == 0:
                    emit_h8(0)
                h8_sb = h8_tiles.pop(img)

                # ================= v projection (NC layout: vT8, fp8 DR) ====
                vT8_sb = vp.tile([P, NB, C], f8, tag="vT8")
                for nb in range(NB):
                    ps = psmm.tile([P, FD], f32, tag="mm")
                    for j in range(2):
                        nc.tensor.matmul(
                            ps,
                            h8_sb[:, 2 * j : 2 * j + 2, ts(nb, P)],
                            w_sb["v"][:, 2 * j : 2 * j + 2],
                            start=(j == 0),
                            stop=(j == 1),
                            perf_mode=DR,
                        )
                    # vT8 = fp8(psum/128) = fp8(16*v)
                    nc.scalar.activation(
                        vT8_sb[:, nb], ps, AF.Identity,
                        scale=1.0 / 128.0, bias=zero_sb,
                    )

                if img + 1 < IMGS:
                    load_x(img + 1)
                    gn(img + 1)

                # ================= attention, per query-half =================
                for nh in range(NH):
                    eT8 = epool.tile([P, NB, FD], f8, tag="eT8")
                    # zps accumulates Z/16 broadcast to all 128 partitions
                    # (lhsT = all-ones [128,2,128] fp8); z matmul for pair j
                    # is emitted ~2 chunks late so it never waits on exp.
                    zps = psz.tile([P, FD], f32, tag="z")

                    def zmm(j, eT8=None, zps=None):
                        nc.tensor.matmul(
                            zps, one8_sb, eT8[:, 2 * j : 2 * j + 2],
                            start=(j == 0), stop=(j == NB // 2 - 1),
                            perf_mode=DR,
                        )

                    for nkb in range(NB):
                        drain(1)
                        ps = psmm.tile([P, FD], f32, tag="mm")
                        for kc in range(KC):
                            nc.tensor.matmul(
                                ps,
                                k_sb[:, kc, ts(nkb, P)],
                                q_sb[:, kc, ts(nh, FD)],
                                start=(kc == 0),
                                stop=(kc == KC - 1),
                            )
                        # eT8 = fp8(exp(alpha*s - 4ln2)) = fp8(exp(alpha*s)/16)
                        nc.scalar.activation(
                            eT8[:, nkb], ps, AF.Exp,
                            scale=ALPHA, bias=expb_sb,
                        )
                        if nkb in (3, 5, 7):
                            zmm((nkb - 3) // 2, eT8=eT8, zps=zps)
                    if nh == 0 and img + 1 < IMGS:
                        emit_h8(img + 1)

                    # oT8 = fp8((vT8^T @ eT8) * (1/Z16)) = fp8(16*attn_out)
                    oT8 = opool.tile([P, KC, FD], f8, tag="oT8")
                    r_sb = None
                    for cb in range(KC):
                        ps = psmm.tile([P, FD], f32, tag="mm")
                        for j in range(NB // 2):
                            nc.tensor.matmul(
                                ps,
                                vT8_sb[:, 2 * j : 2 * j + 2, ts(cb, P)],
                                eT8[:, 2 * j : 2 * j + 2],
                                start=(j == 0),
                                stop=(j == NB // 2 - 1),
                                perf_mode=DR,
                            )
                        if cb == 0:
                            zmm(NB // 2 - 1, eT8=eT8, zps=zps)
                            r_sb = rp.tile([P, FD], f32, tag="r")
                            nc.vector.reciprocal_approx_fast(r_sb, zps)
                        # oT8 chunk right away: overlaps the next attn group
                        nc.vector.tensor_mul(oT8[:, cb], ps, r_sb)

                    # final projection (fp8 DR) + bias + residual: deferred
                    def mk_proj(mo, oT8=None, x_sb=None, img=None, nh=None):
                        def go():
                            ps = psmm.tile([P, FD], f32, tag="mm")
                            for j in range(2):
                                nc.tensor.matmul(
                                    ps,
                                    w_sb["p"][:, 2 * j : 2 * j + 2, ts(mo, P)],
                                    oT8[:, 2 * j : 2 * j + 2],
                                    start=(j == 0),
                                    stop=(j == 1),
                                    perf_mode=DR,
                                )
                            t_sb = tp.tile([P, FD], f32, tag="t")
                            if mo % 2 == 0:
                                nc.scalar.activation(
                                    t_sb, ps, AF.Identity,
                                    scale=1.0 / 4096.0,
                                    bias=bp_sb[:, mo : mo + 1],
                                )
                            else:
                                nc.vector.tensor_scalar(
                                    out=t_sb,
                                    in0=ps,
                                    scalar1=1.0 / 4096.0,
                                    scalar2=bp_sb[:, mo : mo + 1],
                                    op0=OP.mult,
                                    op1=OP.add,
                                )
                            nc.vector.tensor_add(
                                t_sb, t_sb, x_sb[:, mo, ts(nh, FD)]
                            )
                            nc.sync.dma_start(
                                out=out_ap[img].rearrange(
                                    "(mo p) n -> p mo n", p=P
                                )[:, mo, ts(nh, FD)],
                                in_=t_sb,
                            )
                        return go

                    for mo in range(KC):
                        pending.append(
                            mk_proj(mo, oT8=oT8, x_sb=x_sb, img=img, nh=nh)
                        )

            drain(len(pending))

    nc.compile()
    return nc


def _get_nc():
    if "nc" not in _CACHE:
        _CACHE["nc"] = build_nc()
    return _CACHE["nc"]


def make_in_maps(x, gn_w, gn_b, wq, bq, wk, bk, wv, bv, wp, bp):
    """Host-side prep: shard batch, pre-transpose weights, lay out biases."""
    import ml_dtypes

    f = np.float32
    f8 = ml_dtypes.float8_e4m3
    x = np.ascontiguousarray(x, dtype=f).reshape(32, C, N)

    def col(v):  # [C] -> [P, KC] with channel (kc*128 + p) at [p, kc]
        return np.ascontiguousarray(np.asarray(v, dtype=f).reshape(KC, P).T)

    wp_f = np.asarray(wp, dtype=f)
    # attention output bias bv commutes through softmax-normalized attn:
    # out = attn@v0 + bv, so proj = wp@out + bp = wp@(attn@v0) + (wp@bv + bp)
    bp_eff = wp_f @ np.asarray(bv, dtype=f) + np.asarray(bp, dtype=f)

    aux = np.zeros((P, 32), dtype=f)
    aux[:, 0:4] = col(bq)
    aux[:, 4:8] = col(bk)
    aux[:, 8:12] = col(bp_eff)
    aux[:, 12:16] = col(gn_w)
    aux[:, 16:20] = col(gn_b)
    aux[:, 20:28] = (np.arange(P)[:, None] // GS == np.arange(G8)[None, :]).astype(f)
    aux[:, 28] = EPS
    aux[:, 29] = -EXP_SHIFT

    common = {
        "wqT": np.ascontiguousarray(np.asarray(wq, dtype=f).T),
        "wkT": np.ascontiguousarray(np.asarray(wk, dtype=f).T),
        "wv8T": np.ascontiguousarray(
            (np.asarray(wv, dtype=f) * S_W).T.astype(f8)
        ),
        "wp8T": np.ascontiguousarray((wp_f * S_W).T.astype(f8)),
        "aux": aux,
        "gt8": np.ascontiguousarray(
            (np.arange(P)[None, :] // GS == np.arange(P)[:, None] % G8).astype(f)
            * (np.arange(P) < G8)[:, None]
        ),
        "ones8": np.ones((P, 2 * P), dtype=f8),
    }
    return [
        {"x": np.ascontiguousarray(x[i * IMGS : (i + 1) * IMGS]), **common}
        for i in range(NCORES)
    ]


def kernel(**inputs):
    from concourse.bass_utils import run_bass_kernel_spmd

    nc = _get_nc()
    in_maps = make_in_maps(**inputs)
    res = run_bass_kernel_spmd(nc, in_maps, list(range(NCORES)))
    out = np.concatenate([res.results[i]["out"] for i in range(NCORES)], axis=0)
    return out.reshape(32, C, 32, 32)
